# revision 8
# baseline (speedup 1.0000x reference)
"""Trainium2 Bass kernel for nn_CoDy (GCN+GRU dynamics model).

Sharding: pure data-parallel over batch B=256 across 8 cores (32 batch each).
Per core, all tensors use a feature-major layout: features (and, for GCN edge
MLPs, sender-object j x feature) on SBUF partitions, (batch, object) on the
free dimension. K=4 objects * 32 local batch = 128 free columns per timestep.

GCN edge MLPs are packed as (j, feat) on partitions (4*32=128) so the
per-pair MLP becomes block-diagonal matmuls and the sum-over-senders folds
into the node-layer matmul (weights tiled over j on the host).

The 149-step autoregressive rollout is fully unrolled; the decoder GRU and
decoder GCN are emitted interleaved with rollout steps so the Tile scheduler
hides them in the rollout's dependency-chain bubbles.
"""

import os
import sys
from contextlib import ExitStack

import numpy as np

for _p in ("/opt/trn_rl_repo", os.path.expanduser("~/.axon_site/_ro/trn_rl_repo")):
    if os.path.isdir(_p) and _p not in sys.path:
        sys.path.insert(0, _p)

import ml_dtypes  # noqa: E402
import concourse.bass as bass  # noqa: E402
import concourse.tile as tile  # noqa: E402
from concourse import bacc, mybir  # noqa: E402
from concourse.alu_op_type import AluOpType as OP  # noqa: E402
from concourse.bass_utils import run_bass_kernel_spmd  # noqa: E402

AF = mybir.ActivationFunctionType
F32 = mybir.dt.float32

NCORES = 8
S, CF, HID, EMB, Z, K = 14, 32, 32, 32, 64, 4
BLOC = 32          # batch per core
BK = BLOC * K      # 128 free columns per timestep

DT_STR = os.environ.get("KDT", "bf16")   # 'bf16' or 'f32'


def _np(x):
    return np.asarray(x, dtype=np.float32)


def _tile4_cols(w):
    # [rows, 32] -> [rows, 128] (weights repeated for each sender block j)
    return np.tile(w, (1, 4))


def _tile4_rows(w):
    # [32, cols] -> [128, cols]
    return np.tile(w, (4, 1))


def _bd4(w):
    # [32,32] -> block_diag(w,w,w,w) [128,128]
    out = np.zeros((128, 128), np.float32)
    for j in range(4):
        out[j * 32:(j + 1) * 32, j * 32:(j + 1) * 32] = w
    return out


class Packer:
    def __init__(self):
        self.cols = 0
        self.slots = {}
        self.arrs = []

    def add(self, name, arr):
        arr = _np(arr)
        assert arr.ndim == 2 and arr.shape[0] <= 128
        self.slots[name] = (self.cols, arr.shape[0], arr.shape[1])
        self.arrs.append(arr)
        self.cols += arr.shape[1]

    def blob(self, np_dtype):
        out = np.zeros((128, self.cols), np.float32)
        for (name, (c0, r, c)), a in zip(self.slots.items(), self.arrs):
            out[:r, c0:c0 + c] = a
        return np.ascontiguousarray(out.astype(np_dtype))


def pack_params(params):
    """Host-side packing of all weights (wp, dtype DT) and biases (bp, fp32)."""
    wp, bp = Packer(), Packer()

    def gru(tag, layers):
        for li, lyr in enumerate(layers):
            wih = _np(lyr["wih"])  # [3h, in]
            whh = _np(lyr["whh"])  # [3h, h]
            bih = _np(lyr["bih"])
            bhh = _np(lyr["bhh"])
            h = whh.shape[1]
            wp.add(f"{tag}{li}_wih", wih.T)   # [in, 3h]
            wp.add(f"{tag}{li}_whh", whh.T)   # [h, 3h]
            bp.add(f"{tag}{li}_brz_r", (bih[:h] + bhh[:h]).reshape(-1, 1))
            bp.add(f"{tag}{li}_brz_z", (bih[h:2 * h] + bhh[h:2 * h]).reshape(-1, 1))
            bp.add(f"{tag}{li}_bihn", bih[2 * h:].reshape(-1, 1))
            bp.add(f"{tag}{li}_bhhn", bhh[2 * h:].reshape(-1, 1))

    # ---- conf_gcn: S -> EMB, 2 extra edge layers, hid 32
    g = params["conf_gcn"]
    w1 = _np(g["edge"][0]["w"])  # [28, 32]
    wp.add("c_e1i", _tile4_cols(w1[:S]))
    wp.add("c_e1j", w1[S:2 * S])
    bp.add("c_b1", np.tile(_np(g["edge"][0]["b"]), 4).reshape(128, 1))
    for li in (1, 2):
        wp.add(f"c_e{li+1}bd", _bd4(_np(g["edge"][li]["w"])))
        bp.add(f"c_b{li+1}", np.tile(_np(g["edge"][li]["b"]), 4).reshape(128, 1))
    n0 = _np(g["node"][0]["w"])  # [46, 32]
    wp.add("c_n0x", n0[:S])
    wp.add("c_n0m", _tile4_rows(n0[S:S + HID]))
    bp.add("c_n0b", _np(g["node"][0]["b"]).reshape(-1, 1))
    wp.add("c_n1", _np(g["node"][1]["w"]))  # [32, 32]
    bp.add("c_n1b", _np(g["node"][1]["b"]).reshape(-1, 1))

    gru("cg", params["conf_gru"])

    # ---- enc_gcn: S -> Z, 0 extra edge layers
    g = params["enc_gcn"]
    w1 = _np(g["edge"][0]["w"])
    wp.add("e_e1i", _tile4_cols(w1[:S]))
    wp.add("e_e1j", w1[S:2 * S])
    bp.add("e_b1", np.tile(_np(g["edge"][0]["b"]), 4).reshape(128, 1))
    n0 = _np(g["node"][0]["w"])  # [46, 32]
    wp.add("e_n0x", n0[:S])
    wp.add("e_n0m", _tile4_rows(n0[S:S + HID]))
    bp.add("e_n0b", _np(g["node"][0]["b"]).reshape(-1, 1))
    wp.add("e_n1", _np(g["node"][1]["w"]))  # [32, 64]
    bp.add("e_n1b", _np(g["node"][1]["b"]).reshape(-1, 1))

    # ---- dyn_gcn: (Z+CF) -> EMB, 2 extra edge layers
    g = params["dyn_gcn"]
    w1 = _np(g["edge"][0]["w"])  # [192, 32]
    D = Z + CF  # 96
    wp.add("d_e1i_z", _tile4_cols(w1[:Z]))
    wp.add("d_e1i_c", _tile4_cols(w1[Z:D]))
    wp.add("d_e1j_z", w1[D:D + Z])
    wp.add("d_e1j_c", w1[D + Z:2 * D])
    bp.add("d_b1", np.tile(_np(g["edge"][0]["b"]), 4).reshape(128, 1))
    for li in (1, 2):
        wp.add(f"d_e{li+1}bd", _bd4(_np(g["edge"][li]["w"])))
        bp.add(f"d_b{li+1}", np.tile(_np(g["edge"][li]["b"]), 4).reshape(128, 1))
    n0 = _np(g["node"][0]["w"])  # [128, 32]
    wp.add("d_n0z", n0[:Z])
    wp.add("d_n0c", n0[Z:D])
    wp.add("d_n0m", _tile4_rows(n0[D:D + HID]))
    bp.add("d_n0b", _np(g["node"][0]["b"]).reshape(-1, 1))
    wp.add("d_n1", _np(g["node"][1]["w"]))  # [32, 32]
    bp.add("d_n1b", _np(g["node"][1]["b"]).reshape(-1, 1))

    gru("dg", params["dyn_gru"])
    wp.add("dynlin", _np(params["dyn_lin_w"]))  # [32, 64]

    gru("xg", params["dec_gru"])  # 1 layer, in 64, h 64

    # ---- dec_gcn: Z -> S, 0 extra edge layers
    g = params["dec_gcn"]
    w1 = _np(g["edge"][0]["w"])  # [128, 32]
    wp.add("g_e1i", _tile4_cols(w1[:Z]))
    wp.add("g_e1j", w1[Z:2 * Z])
    bp.add("g_b1", np.tile(_np(g["edge"][0]["b"]), 4).reshape(128, 1))
    n0 = _np(g["node"][0]["w"])  # [96, 32]
    wp.add("g_n0x", n0[:Z])
    wp.add("g_n0m", _tile4_rows(n0[Z:Z + HID]))
    bp.add("g_n0b", _np(g["node"][0]["b"]).reshape(-1, 1))
    wp.add("g_n1", _np(g["node"][1]["w"]))  # [32, 14]
    bp.add("g_n1b", _np(g["node"][1]["b"]).reshape(-1, 1))

    wp.add("I128", np.eye(128, dtype=np.float32))
    wp.add("I32", np.eye(32, dtype=np.float32))
    return wp, bp


def build_program(t_ab, horizon, dt_str=DT_STR, wp_slots=None, bp_slots=None,
                  phases=("conf", "roll", "dec")):
    """Emit the full SPMD program. Returns the compiled Bacc object."""
    DT = mybir.dt.bfloat16 if dt_str == "bf16" else F32
    r32 = dt_str != "bf16"
    T = horizon + 1

    nc = bacc.Bacc("TRN2", target_bir_lowering=False, debug=False,
                   num_devices=NCORES)

    nw = sum(c for (_, _, c) in wp_slots.values())
    nb = sum(c for (_, _, c) in bp_slots.values())
    BF = mybir.dt.bfloat16
    abT_d = nc.dram_tensor("abT", [S, t_ab * BK], BF, kind="ExternalInput").ap()
    cT_d = nc.dram_tensor("cT", [S, BK], BF, kind="ExternalInput").ap()
    wb_d = nc.dram_tensor("wb", [128, nw], DT, kind="ExternalInput").ap()
    wbh_d = nc.dram_tensor("wbh", [128, nw], mybir.dt.bfloat16,
                           kind="ExternalInput").ap()
    bb_d = nc.dram_tensor("bb", [128, nb], F32, kind="ExternalInput").ap()
    out_d = nc.dram_tensor("out", [S, T * BK], F32, kind="ExternalOutput").ap()

    with ExitStack() as ctx:
        tc = ctx.enter_context(tile.TileContext(nc))
        per = ctx.enter_context(tc.tile_pool(name="per", bufs=1))
        sb = ctx.enter_context(tc.tile_pool(name="sb", bufs=3))
        st = ctx.enter_context(tc.tile_pool(name="st", bufs=2))
        psE = ctx.enter_context(tc.tile_pool(name="psE", bufs=2, space="PSUM"))
        psG = ctx.enter_context(tc.tile_pool(name="psG", bufs=2, space="PSUM"))
        psB = ctx.enter_context(tc.tile_pool(name="psB", bufs=4, space="PSUM"))

        wt = per.tile([128, nw], DT, tag="wt")
        nc.sync.dma_start(wt[:], wb_d[:])
        wth = per.tile([128, nw], mybir.dt.bfloat16, tag="wth")
        nc.sync.dma_start(wth[:], wbh_d[:])
        bt = per.tile([128, nb], F32, tag="bt")
        nc.sync.dma_start(bt[:], bb_d[:])

        def W(name):
            c0, r, c = wp_slots[name]
            return wt[0:r, c0:c0 + c]

        def Wh(name):
            c0, r, c = wp_slots[name]
            return wth[0:r, c0:c0 + c]

        def Bi(name):
            c0, r, c = bp_slots[name]
            return bt[0:r, c0:c0 + c]

        def mm(ps, lhsT, rhs, start, stop, tp=None):
            nc.tensor.matmul(ps, lhsT, rhs, start=start, stop=stop,
                             skip_group_check=True, tile_position=tp)

        def edge1(ps, wi, wj, x, Din):
            """First GCN edge layer into psum [(j,f)=128, Fr]; x [Din, Fr]."""
            mm(ps, wi, x, start=True, stop=False)
            nf = x.shape[1] // 4
            xv = x.rearrange("p (f k) -> p f k", k=4)
            for j in range(4):
                rhs = xv[:, :, j].unsqueeze(2).broadcast_to([Din, nf, 4])
                mm(ps[j * 32:(j + 1) * 32, :], wj, rhs, start=False, stop=True,
                   tp=(0, j * 32))

        x_conf = per.tile([S, t_ab * BK], mybir.dt.bfloat16, tag="x_conf")
        nc.sync.dma_start(x_conf[:], abT_d[:])
        cT = per.tile([S, BK], mybir.dt.bfloat16, tag="cT")
        nc.sync.dma_start(cT[:], cT_d[:])

        gru_in = per.tile([HID, t_ab * BK], mybir.dt.bfloat16, tag="gru_in")
        z_seq = per.tile([Z, T * BK], DT, tag="z_seq")
        decout = per.tile([Z, T * BK], mybir.dt.bfloat16, tag="decout")

        # ================= conf GCN (throughput) =================
        frames = t_ab * BLOC
        fr0 = 0
        while ("conf" in phases) and fr0 < frames:
            nf = min(128, frames - fr0)
            Fr = nf * 4
            x = x_conf[:, fr0 * 4: fr0 * 4 + Fr]
            ps_e = psG.tile([128, Fr], F32, tag="pg")
            edge1(ps_e, Wh("c_e1i"), Wh("c_e1j"), x, S)
            e1 = sb.tile([128, Fr], mybir.dt.bfloat16, tag="ge1")
            nc.scalar.activation(e1[:], ps_e[:], AF.Relu, bias=Bi("c_b1"))
            ps_e2 = psG.tile([128, Fr], F32, tag="pg")
            mm(ps_e2[:], Wh("c_e2bd"), e1[:], start=True, stop=True)
            e2 = sb.tile([128, Fr], mybir.dt.bfloat16, tag="ge2")
            nc.scalar.activation(e2[:], ps_e2[:], AF.Relu, bias=Bi("c_b2"))
            ps_e3 = psG.tile([128, Fr], F32, tag="pg")
            mm(ps_e3[:], Wh("c_e3bd"), e2[:], start=True, stop=True)
            e3 = sb.tile([128, Fr], mybir.dt.bfloat16, tag="ge1")
            nc.scalar.activation(e3[:], ps_e3[:], AF.Relu, bias=Bi("c_b3"))
            ps_n = psG.tile([HID, Fr], F32, tag="pg")
            mm(ps_n[:], Wh("c_n0m"), e3[:], start=True, stop=False)
            mm(ps_n[:], Wh("c_n0x"), x, start=False, stop=True)
            hx = sb.tile([HID, Fr], mybir.dt.bfloat16, tag="ge2")
            nc.scalar.activation(hx[:], ps_n[:], AF.Relu, bias=Bi("c_n0b"))
            ps_o = psG.tile([EMB, Fr], F32, tag="pg")
            mm(ps_o[:], Wh("c_n1"), hx[:], start=True, stop=True)
            nc.scalar.activation(gru_in[:, fr0 * 4: fr0 * 4 + Fr], ps_o[:],
                                 AF.Identity, bias=Bi("c_n1b"))
            fr0 += nf

        # ================= GRU cell =================
        def gru_cell(tag, x_ap, h_ap, hdim, newh_ap=None, xh_bf=False):
            Wx = Wh if xh_bf else W
            h2d = 2 * hdim
            ps_rz = psB.tile([h2d, BK], F32, tag="pb")
            mm(ps_rz[:], W(f"{tag}_whh")[:, 0:h2d], h_ap, start=True, stop=False)
            mm(ps_rz[:], Wx(f"{tag}_wih")[:, 0:h2d], x_ap, start=False, stop=True)
            r = sb.tile([hdim, BK], DT, tag="rz")
            nc.scalar.activation(r[:], ps_rz[0:hdim, :], AF.Sigmoid,
                                 bias=Bi(f"{tag}_brz_r"))
            zg = sb.tile([hdim, BK], DT, tag="zg")
            nc.scalar.activation(zg[:], ps_rz[hdim:h2d, :], AF.Sigmoid,
                                 bias=Bi(f"{tag}_brz_z"))
            ps_hn = psB.tile([hdim, BK], F32, tag="pb")
            mm(ps_hn[:], W(f"{tag}_whh")[:, h2d:3 * hdim], h_ap, start=True, stop=True)
            ps_in = psB.tile([hdim, BK], F32, tag="pb")
            mm(ps_in[:], Wx(f"{tag}_wih")[:, h2d:3 * hdim], x_ap, start=True, stop=True)
            t2 = sb.tile([hdim, BK], DT, tag="t2")
            nc.vector.scalar_tensor_tensor(t2[:], ps_hn[:], Bi(f"{tag}_bhhn"),
                                           r[:], op0=OP.add, op1=OP.mult)
            u = sb.tile([hdim, BK], F32, tag="u")
            nc.vector.tensor_tensor(u[:], t2[:], ps_in[:], op=OP.add)
            n = sb.tile([hdim, BK], DT, tag="n")
            nc.scalar.activation(n[:], u[:], AF.Tanh, bias=Bi(f"{tag}_bihn"))
            q = sb.tile([hdim, BK], DT, tag="q")
            nc.vector.tensor_tensor(q[:], h_ap, n[:], op=OP.subtract)
            p = sb.tile([hdim, BK], DT, tag="p")
            nc.vector.tensor_tensor(p[:], q[:], zg[:], op=OP.mult)
            if newh_ap is None:
                nh = st.tile([hdim, BK], DT, tag=f"h_{tag}")
                newh_ap = nh[:]
            nc.vector.tensor_tensor(newh_ap, p[:], n[:], op=OP.add)
            return newh_ap

        # ================= conf GRU (30 steps, 2 layers) =================
        hc1 = st.tile([CF, BK], DT, tag="h_cg0")
        nc.gpsimd.memset(hc1[:], 0.0)
        hc2 = st.tile([CF, BK], DT, tag="h_cg1")
        nc.gpsimd.memset(hc2[:], 0.0)
        hc1_ap, hc2_ap = hc1[:], hc2[:]
        if "conf" in phases:
            for t in range(t_ab):
                xt = gru_in[:, t * BK:(t + 1) * BK]
                hc1_ap = gru_cell("cg0", xt, hc1_ap, CF, xh_bf=True)
                hc2_ap = gru_cell("cg1", hc1_ap, hc2_ap, CF)
        cf = per.tile([CF, BK], DT, tag="cf")
        if "conf" in phases:
            nc.vector.tensor_copy(cf[:], hc2_ap)
        else:
            nc.gpsimd.memset(cf[:], 0.01)

        # ================= encoder GCN =================
        ps_e = psG.tile([128, BK], F32, tag="pg")
        edge1(ps_e, Wh("e_e1i"), Wh("e_e1j"), cT[:], S)
        ee1 = sb.tile([128, BK], mybir.dt.bfloat16, tag="ge1")
        nc.scalar.activation(ee1[:], ps_e[:], AF.Relu, bias=Bi("e_b1"))
        ps_n = psG.tile([HID, BK], F32, tag="pg")
        mm(ps_n[:], Wh("e_n0m"), ee1[:], start=True, stop=False)
        mm(ps_n[:], Wh("e_n0x"), cT[:], start=False, stop=True)
        eh = sb.tile([HID, BK], mybir.dt.bfloat16, tag="ge2")
        nc.scalar.activation(eh[:], ps_n[:], AF.Relu, bias=Bi("e_n0b"))
        ps_z = psG.tile([Z, BK], F32, tag="pg")
        mm(ps_z[:], Wh("e_n1"), eh[:], start=True, stop=True)
        if dt_str == "bf16":
            z_fp = st.tile([Z, BK], F32, tag="zfp")
            nc.scalar.activation(z_fp[:], ps_z[:], AF.Identity, bias=Bi("e_n1b"))
            nc.scalar.copy(z_seq[:, 0:BK], z_fp[:])
            z_fp_ap = z_fp[:]
        else:
            nc.scalar.activation(z_seq[:, 0:BK], ps_z[:], AF.Identity, bias=Bi("e_n1b"))
            z_fp_ap = z_seq[:, 0:BK]

        # ================= cf-constant precompute for dyn GCN ============
        ps_c = psG.tile([128, BK], F32, tag="pg")
        edge1(ps_c, W("d_e1i_c"), W("d_e1j_c"), cf[:], CF)
        cfe = per.tile([128, BK], DT, tag="cfe")
        nc.scalar.activation(cfe[:], ps_c[:], AF.Identity, bias=Bi("d_b1"))
        ps_c2 = psG.tile([HID, BK], F32, tag="pg")
        mm(ps_c2[:], W("d_n0c"), cf[:], start=True, stop=True)
        cfn = per.tile([HID, BK], DT, tag="cfn")
        nc.scalar.activation(cfn[:], ps_c2[:], AF.Identity, bias=Bi("d_n0b"))

        # ================= decoder step =================
        hd0 = st.tile([Z, BK], DT, tag="h_dec")
        nc.gpsimd.memset(hd0[:], 0.0)
        dec_h = [hd0[:]]

        def dec_step(tau):
            zb = z_seq[:, tau * BK:(tau + 1) * BK]
            if dt_str == "bf16":
                newh = decout[:, tau * BK:(tau + 1) * BK]
                dec_h[0] = gru_cell("xg0", zb, dec_h[0], Z, newh_ap=newh)
            else:
                dec_h[0] = gru_cell("xg0", zb, dec_h[0], Z)
                nc.scalar.copy(decout[:, tau * BK:(tau + 1) * BK], dec_h[0])

        def dec_gcn(tau0, ntau):
            Fr = ntau * BK
            x = decout[:, tau0 * BK: tau0 * BK + Fr]
            ps_e = psG.tile([128, Fr], F32, tag="pg")
            nc.tensor.matmul(ps_e[:], Wh("g_e1i"), x, start=True, stop=False,
                             skip_group_check=True)
            nf = Fr // 4
            xv = x.rearrange("p (f k) -> p f k", k=4)
            for j in range(4):
                rhs = xv[:, :, j].unsqueeze(2).broadcast_to([Z, nf, 4])
                nc.tensor.matmul(ps_e[j * 32:(j + 1) * 32, :], Wh("g_e1j"), rhs,
                                 start=False, stop=True, skip_group_check=True,
                                 tile_position=(0, j * 32))
            ge1 = sb.tile([128, Fr], mybir.dt.bfloat16, tag="ge1")
            nc.scalar.activation(ge1[:], ps_e[:], AF.Relu, bias=Bi("g_b1"))
            ps_n = psG.tile([HID, Fr], F32, tag="pg")
            nc.tensor.matmul(ps_n[:], Wh("g_n0m"), ge1[:], start=True, stop=False,
                             skip_group_check=True)
            nc.tensor.matmul(ps_n[:], Wh("g_n0x"), x, start=False, stop=True,
                             skip_group_check=True)
            gh = sb.tile([HID, Fr], mybir.dt.bfloat16, tag="ge2")
            nc.scalar.activation(gh[:], ps_n[:], AF.Relu, bias=Bi("g_n0b"))
            ps_o = psG.tile([S, Fr], F32, tag="pg")
            nc.tensor.matmul(ps_o[:], Wh("g_n1"), gh[:], start=True, stop=True,
                             skip_group_check=True)
            o = sb.tile([S, Fr], F32, tag="oout")
            nc.scalar.activation(o[:], ps_o[:], AF.Identity, bias=Bi("g_n1b"))
            nc.sync.dma_start(out_d[:, tau0 * BK: tau0 * BK + Fr], o[:])

        if "dec" in phases:
            dec_step(0)

        # ================= rollout (horizon steps) =================
        h1 = st.tile([EMB, BK], DT, tag="h_dg0")
        nc.gpsimd.memset(h1[:], 0.0)
        h2 = st.tile([EMB, BK], DT, tag="h_dg1")
        nc.gpsimd.memset(h2[:], 0.0)
        h1_ap, h2_ap = h1[:], h2[:]
        dec_done = 1

        if "roll" not in phases:
            horizon_eff = 0
            nc.gpsimd.memset(z_seq[:], 0.01)
        else:
            horizon_eff = horizon
        for t in range(horizon_eff):
            zb = z_seq[:, t * BK:(t + 1) * BK]
            ps_e = psE.tile([128, BK], F32, tag="pe")
            mm(ps_e[:], W("I128"), cfe[:], start=True, stop=False)
            mm(ps_e[:], W("d_e1i_z"), zb, start=False, stop=False)
            zv = zb.rearrange("p (b k) -> p b k", k=4)
            for j in range(4):
                rhs = zv[:, :, j].unsqueeze(2).broadcast_to([Z, BLOC, 4])
                mm(ps_e[j * 32:(j + 1) * 32, :], W("d_e1j_z"), rhs,
                   start=False, stop=True, tp=(0, j * 32))
            de1 = sb.tile([128, BK], DT, tag="de1")
            nc.scalar.activation(de1[:], ps_e[:], AF.Relu)
            ps_e2 = psE.tile([128, BK], F32, tag="pe")
            mm(ps_e2[:], W("d_e2bd"), de1[:], start=True, stop=True)
            de2 = sb.tile([128, BK], DT, tag="de2")
            nc.scalar.activation(de2[:], ps_e2[:], AF.Relu, bias=Bi("d_b2"))
            ps_e3 = psE.tile([128, BK], F32, tag="pe")
            mm(ps_e3[:], W("d_e3bd"), de2[:], start=True, stop=True)
            de3 = sb.tile([128, BK], DT, tag="de1")
            nc.scalar.activation(de3[:], ps_e3[:], AF.Relu, bias=Bi("d_b3"))
            ps_n = psB.tile([HID, BK], F32, tag="pb")
            mm(ps_n[:], W("d_n0m"), de3[:], start=True, stop=False)
            mm(ps_n[:], W("d_n0z"), zb, start=False, stop=False)
            mm(ps_n[:], W("I32"), cfn[:], start=False, stop=True)
            dh = sb.tile([HID, BK], DT, tag="de2")
            nc.scalar.activation(dh[:], ps_n[:], AF.Relu)
            ps_o = psB.tile([EMB, BK], F32, tag="pb")
            mm(ps_o[:], W("d_n1"), dh[:], start=True, stop=True)
            eg = sb.tile([EMB, BK], DT, tag="eg")
            nc.scalar.activation(eg[:], ps_o[:], AF.Identity, bias=Bi("d_n1b"))

            h1_ap = gru_cell("dg0", eg[:], h1_ap, EMB)
            h2_ap = gru_cell("dg1", h1_ap, h2_ap, EMB)

            ps_d = psB.tile([Z, BK], F32, tag="pb")
            mm(ps_d[:], W("dynlin"), h2_ap, start=True, stop=True)
            if dt_str == "bf16":
                zn = st.tile([Z, BK], F32, tag="zfp")
                nc.vector.tensor_tensor(zn[:], z_fp_ap, ps_d[:], op=OP.add)
                nc.scalar.copy(z_seq[:, (t + 1) * BK:(t + 2) * BK], zn[:])
                z_fp_ap = zn[:]
            else:
                zslot = z_seq[:, (t + 1) * BK:(t + 2) * BK]
                nc.vector.tensor_tensor(zslot, z_fp_ap, ps_d[:], op=OP.add)
                z_fp_ap = zslot

            if "dec" in phases:
                dec_step(t + 1)
            dec_done += 1
            if dec_done % 4 == 0 and "dec" in phases:
                dec_gcn(dec_done - 4, 4)

        if "dec" in phases and "roll" not in phases:
            for t in range(horizon):
                dec_step(t + 1)
                dec_done += 1
                if dec_done % 4 == 0:
                    dec_gcn(dec_done - 4, 4)
        if dec_done % 4 != 0 and "dec" in phases:
            tau0 = dec_done - (dec_done % 4)
            dec_gcn(tau0, dec_done - tau0)

    nc.compile()
    return nc


_CACHE = {}


def _get_program(t_ab, horizon, dt_str, params):
    key = (t_ab, horizon, dt_str)
    if key not in _CACHE:
        wp, bp = pack_params(params)
        nc = build_program(t_ab, horizon, dt_str, wp.slots, bp.slots)
        _CACHE[key] = (nc, wp.slots, bp.slots)
    return _CACHE[key]


def kernel(ab, c, params, horizon):
    ab = _np(ab)
    c = _np(c)
    horizon = int(horizon)
    B, t_ab = ab.shape[0], ab.shape[1]
    T = horizon + 1
    assert B % NCORES == 0
    bl = B // NCORES
    assert bl == BLOC, f"expected {BLOC} batch per core, got {bl}"

    np_dt = ml_dtypes.bfloat16 if DT_STR == "bf16" else np.float32
    nc, wslots, bslots = _get_program(t_ab, horizon, DT_STR, params)

    wp, bp = pack_params(params)
    wb = wp.blob(np_dt)
    wbh = wp.blob(ml_dtypes.bfloat16)
    bb = bp.blob(np.float32)

    in_maps = []
    for ci in range(NCORES):
        abs_ = ab[ci * bl:(ci + 1) * bl]              # [32, t_ab, 4, 14]
        cs = c[ci * bl:(ci + 1) * bl]                 # [32, 4, 14]
        abT = np.ascontiguousarray(
            abs_.transpose(3, 1, 0, 2).reshape(S, t_ab * BK).astype(ml_dtypes.bfloat16))
        cT = np.ascontiguousarray(
            cs.transpose(2, 0, 1).reshape(S, BK).astype(ml_dtypes.bfloat16))
        in_maps.append({"abT": abT, "cT": cT, "wb": wb, "wbh": wbh, "bb": bb})

    res = run_bass_kernel_spmd(nc, in_maps, core_ids=list(range(NCORES)))

    out = np.empty((B, T, K, S), np.float32)
    for ci in range(NCORES):
        o = res.results[ci]["out"]                    # [14, T*128]
        o = o.reshape(S, T, bl, K).transpose(2, 1, 3, 0)
        out[ci * bl:(ci + 1) * bl] = o
    return out
